# revision 20
# baseline (speedup 1.0000x reference)
"""Multi-head attention (B=2, S=2048, D=1024, H=16) on 8 TRN2 NeuronCores.

Sharding: tensor-parallel over heads x data-parallel over batch.
Core c handles batch b = c // 4 and head group g = c % 4 (4 heads each).
Each core computes its 4 heads' q/k/v projections, attention, and the
partial output projection against its slice of Wo; the host sums the 4
partials per batch element.

Per-core kernel layout:
  - inputs: xT [1024, 2048] (= x[b].T), wq/wk/wv [1024, 256] (= W[rows].T),
    wo [256, 1024] (= Wo[:, cols].T)
  - QT/KT/VT computed transposed ([head-feat, seq]) so the Dh-contraction
    of q@k^T has its contraction dim on partitions.
  - scores are computed transposed ([keys, q]) for a head PAIR into one
    2-bank psum tile; one wide exp via ACT (scale folded); attn @ v
    contracts keys on partitions; columns 64:128 of the v operand hold
    ones so the same matmul emits softmax row-sums replicated across 64
    psum rows (partition-broadcast APs are illegal on DVE, replicating in
    the matmul is free).

fp16 streaming: matmul operands are fp16 (1 cyc/row on the PE vs ~1.6 for
f32r, half the LDWEIGHTS and SBUF cost) while every accumulation stays
f32 in PSUM. Value ranges fit fp16 comfortably (|q|,|k| ~ N(0,1),
exp(scores*scale) <= ~e^7; fp16 max is 65504).

Scheduling: the Tile scheduler is greedy per engine with emission order
as priority, so emission order software-pipelines the phases:
  warmup/DMA -> proj pair0 (k, v+transpose, q chunk0) -> attention pair0
  chunk0 -> q chunks 1-3 -> attention pair0 1-3 -> proj pair1 ->
  attention pair1 -> output projection.
Attention is ACT(exp)-bound (~1.1us per [128,1024] exp tile, 128 tiles);
emitting attention pair0 before proj pair1 starts the exp stream ~45us
earlier and the later-priority proj/outproj matmuls fill the PE's
exp-wait gaps. ACT runs exps exclusively; all psum evacuations are
pinned to DVE.
"""

import numpy as np

B, S, D, H, DH = 2, 2048, 1024, 16, 64
NCORES = 8
GROUPS = 4  # head groups; 4 heads = 256 features per core
M = 256  # head features per core
SCALE = 0.125  # 1/sqrt(64)

# stream dtypes per matmul group: "f32r", "bf16", or "fp16"
CFG = {
    "proj": "fp16",   # xT, wq/wk/wv
    "scores": "fp16",  # QT, KT
    "av": "fp16",      # VA, exp tiles
    "wo": "fp16",      # OT, wo
}

_compiled = None


def _dt(mybir, name):
    return {"f32r": mybir.dt.float32r, "bf16": mybir.dt.bfloat16,
            "fp16": mybir.dt.float16}[name]


def _np_dt(name):
    if name == "bf16":
        import ml_dtypes
        return ml_dtypes.bfloat16
    if name == "fp16":
        return np.float16
    return np.float32


def _build_module():
    import concourse.mybir as mybir
    import concourse.tile as tile
    from concourse import bacc

    f32 = mybir.dt.float32
    in_dt = _dt(mybir, CFG["proj"])
    wo_dt = _dt(mybir, CFG["wo"])
    nc = bacc.Bacc("TRN2", target_bir_lowering=False, debug=False,
                   num_devices=NCORES)
    xT = nc.dram_tensor("xT", [D, S], in_dt, kind="ExternalInput").ap()
    wq = nc.dram_tensor("wq", [D, M], in_dt, kind="ExternalInput").ap()
    wk = nc.dram_tensor("wk", [D, M], in_dt, kind="ExternalInput").ap()
    wv = nc.dram_tensor("wv", [D, M], in_dt, kind="ExternalInput").ap()
    wo = nc.dram_tensor("wo", [M, D], wo_dt, kind="ExternalInput").ap()
    out = nc.dram_tensor("out", [S, D], f32, kind="ExternalOutput").ap()

    with tile.TileContext(nc) as tc:
        _kernel_body(tc, out, xT, wq, wk, wv, wo)
    nc.compile()
    return nc


def _kernel_body(tc, out, xT, wq, wk, wv, wo):
    from contextlib import ExitStack

    import concourse.mybir as mybir
    from concourse.masks import make_identity

    nc = tc.nc
    f32 = mybir.dt.float32
    f32r = mybir.dt.float32r
    sc_dt = _dt(mybir, CFG["scores"])
    av_dt = _dt(mybir, CFG["av"])
    wo_dt = _dt(mybir, CFG["wo"])
    AF = mybir.ActivationFunctionType

    P = 128
    NKT = D // P   # 8 k-tiles in the projection contraction
    NPT = M // P   # 2 partition-tiles of head features
    SKT = S // P   # 16 key tiles
    QC = 512       # q chunk (psum bank width in f32)
    NQC = S // QC  # 4
    HPC = 4        # heads per core

    with ExitStack() as ctx:
        const = ctx.enter_context(tc.tile_pool(name="const", bufs=1))
        big = ctx.enter_context(tc.tile_pool(name="big", bufs=1))
        wpool = ctx.enter_context(tc.tile_pool(name="w", bufs=1))
        projin = ctx.enter_context(tc.tile_pool(name="projin", bufs=1))
        work = ctx.enter_context(tc.tile_pool(name="work", bufs=2))
        exp_pool = ctx.enter_context(tc.tile_pool(name="exp", bufs=8))
        small = ctx.enter_context(tc.tile_pool(name="small", bufs=2))
        # PSUM budget (8 banks): psA 2 + psS 2x2 + psO 2x1 = 8
        psum_big = ctx.enter_context(tc.tile_pool(name="psA", bufs=2, space="PSUM"))
        psum_s = ctx.enter_context(tc.tile_pool(name="psS", bufs=2, space="PSUM"))
        psum_o = ctx.enter_context(tc.tile_pool(name="psO", bufs=1, space="PSUM"))

        # warm the PE clock (HAM): dummy matmuls on a DVE-memset tile keep
        # the activity monitor busy during the input DMA head so the real
        # projections start at full clock. The memset tile needs no
        # identity, so warmup starts as soon as the DVE is free (~5us).
        wsrc = const.tile([P, P], f32, tag="wsrc")
        nc.vector.memset(wsrc[:], 1.0)
        wsrc_r = wsrc[:].bitcast(f32r)
        warm_ps = psum_big.tile([P, P], f32, tag="ps_big")
        for _ in range(16):
            nc.tensor.matmul(warm_ps[:], wsrc_r, wsrc_r,
                             start=True, stop=True)

        # fp16 identity: transposes stream at 1.0 cyc/row (vs 1.5 f32r)
        ident = const.tile([P, P], av_dt, tag="ident_h")
        make_identity(nc, ident)

        QT = big.tile([P, NPT, S], sc_dt, tag="QT")
        KT = big.tile([P, NPT, S], sc_dt, tag="KT")
        VT = big.tile([P, NPT, S], av_dt, tag="VT")
        proj_dst = {"q": QT, "k": KT, "v": VT}
        OT = big.tile([P, NPT, S], wo_dt, tag="OT")
        VA = big.tile([P, HPC, SKT, P], av_dt, tag="VA")
        wo_sb = wpool.tile([P, NPT, D], wo_dt, tag="wo")

        # --- input DMAs: all on the sync HWDGE queue (SWDGE issue is
        # ~1.1us each and forces expensive gpsimd drains), ordered to
        # match consumption: wk first, then xT chunk 0 finely sliced so
        # the first k-projection fill starts as early as possible. ---
        w_sb = {}
        for name, w in (("k", wk), ("v", wv), ("q", wq)):
            w_sb[name] = projin.tile([P, NKT, M], w.dtype, tag=f"w{name}",
                                     name=f"w{name}")
        xT_sb = projin.tile([P, NKT, S], xT.dtype, tag="xT")
        xT_r = xT.rearrange("(kt p) s -> p kt s", p=P)

        def xt_slices(c, npieces):
            n = NKT // npieces
            for kh in range(npieces):
                nc.sync.dma_start(
                    xT_sb[:, n * kh:n * kh + n, c * QC:(c + 1) * QC],
                    xT_r[:, n * kh:n * kh + n, c * QC:(c + 1) * QC])

        nc.sync.dma_start(w_sb["k"][:], wk.rearrange("(kt p) m -> p kt m", p=P))
        xt_slices(0, 4)
        xt_slices(1, 2)
        xt_slices(2, 2)
        xt_slices(3, 2)
        nc.sync.dma_start(w_sb["v"][:], wv.rearrange("(kt p) m -> p kt m", p=P))
        nc.sync.dma_start(w_sb["q"][:], wq.rearrange("(kt p) m -> p kt m", p=P))
        nc.sync.dma_start(wo_sb[:], wo.rearrange("(pt p) n -> p pt n", p=P))

        # ones block of VA for the softmax row sums; strided memsets on the
        # otherwise-idle gpsimd
        for h in range(HPC):
            nc.gpsimd.memset(VA[:, h, :, 64:128], 1.0)

        # --- projection helpers. Evacuations: ACT is exp-idle before the
        # first attention chunk, so pre-attention fills evacuate there;
        # everything emitted later must stay off ACT (DVE) or it stalls
        # the exp stream. Fills alternate between the psum_big ring and
        # the (attention-idle) psum_s banks for a 4-deep pipeline. ---
        def sc_copy(dst, srcap):
            nc.scalar.copy(dst, srcap)

        def ve_copy(dst, srcap):
            nc.vector.tensor_copy(dst, srcap)

        _fill_flip = [False]

        def proj_fill(name, pt, c, evac, deep=False):
            # deep: pre-attention only — once attention runs, the ps_s ring
            # belongs to the score tiles and a fill slotting into it would
            # force itself ahead of higher-priority scores (ring WAR dep).
            dst = proj_dst[name]
            _fill_flip[0] = not _fill_flip[0]
            if deep and not _fill_flip[0]:
                ps2 = psum_s.tile([P, 2, QC], f32, tag="ps_s")
                ps = ps2[:, 0, :]
            else:
                ps = psum_big.tile([P, QC], f32, tag="ps_big")
            for kt in range(NKT):
                nc.tensor.matmul(
                    ps[:],
                    w_sb[name][:, kt, pt * P:(pt + 1) * P],
                    xT_sb[:, kt, c * QC:(c + 1) * QC],
                    start=(kt == 0), stop=(kt == NKT - 1),
                )
            evac(dst[:, pt, c * QC:(c + 1) * QC], ps[:])

        def v_transposes(pt, c):
            # V back to natural layout for the attn@v contraction; the
            # small VA copies stay on DVE so they don't serialize behind
            # the fill evacuations on ACT
            for st in range(4 * c, 4 * c + 4):
                pst = psum_big.tile([P, P], av_dt, tag="ps_big")
                nc.tensor.transpose(pst[:], VT[:, pt, st * P:(st + 1) * P],
                                    ident)
                nc.vector.tensor_copy(VA[:, 2 * pt, st, 0:64], pst[:, 0:64])
                nc.vector.tensor_copy(VA[:, 2 * pt + 1, st, 0:64],
                                      pst[:, 64:128])

        # --- attention for one (pair, chunk): scores -> exp -> attn@v,
        # then normalize via the replicated row sums ---
        def attn_chunk(p, col0, w=QC):
            cs = slice(col0, col0 + w)
            poA = psum_o.tile([P, w], f32, tag="ps_oA", name="poA")
            poB = psum_o.tile([P, w], f32, tag="ps_oB", name="poB")
            for kt in range(SKT):
                ks = slice(kt * P, (kt + 1) * P)
                # full 2-bank tile even for narrow chunks: each head-half
                # must start on a PSUM bank boundary
                pss = psum_s.tile([P, 2, QC], f32, tag="ps_s", name="pss")
                nc.tensor.matmul(pss[:, 0, 0:w], KT[0:64, p, ks],
                                 QT[0:64, p, cs], start=True, stop=True)
                nc.tensor.matmul(pss[:, 1, 0:w], KT[64:128, p, ks],
                                 QT[64:128, p, cs], start=True, stop=True)
                et = exp_pool.tile([P, 2, w], av_dt, tag="exp", name="et")
                nc.scalar.activation(et[:], pss[:, :, 0:w], AF.Exp, scale=SCALE)
                nc.tensor.matmul(poA[:], VA[:, 2 * p, kt, :], et[:, 0, :],
                                 start=(kt == 0), stop=(kt == SKT - 1))
                nc.tensor.matmul(poB[:], VA[:, 2 * p + 1, kt, :], et[:, 1, :],
                                 start=(kt == 0), stop=(kt == SKT - 1))
            for r0, po in ((0, poA), (64, poB)):
                # sums straight off the psum rows 64:128 (partition-shift
                # copy) so the reciprocal starts immediately; the value
                # rows evacuate in parallel
                sm = small.tile([64, w], f32, tag="sums", name="sm")
                nc.vector.tensor_copy(sm[:], po[64:128, :])
                pc = small.tile([64, w], f32, tag="po_sb", name="pc")
                nc.vector.tensor_copy(pc[:], po[0:64, :])
                rb = small.tile([64, w], f32, tag="recip", name="rb")
                nc.vector.reciprocal_approx_fast(rb[:], sm[:])
                nc.vector.tensor_tensor(
                    OT[r0:r0 + 64, p, cs],
                    pc[:],
                    rb[:],
                    mybir.AluOpType.mult,
                )

        def outproj_chunk(qts, deep=False, evac=None):
            # deep + scalar evac are safe only for the final chunks, when
            # the score psum ring and ACT have gone idle
            evac = evac or ve_copy
            for qt in qts:
                for nch in range(2):
                    _fill_flip[0] = not _fill_flip[0]
                    if deep and not _fill_flip[0]:
                        ps2 = psum_s.tile([P, 2, QC], f32, tag="ps_s")
                        ps = ps2[:, 0, :]
                    else:
                        ps = psum_big.tile([P, 512], f32, tag="ps_big")
                    for pt in range(NPT):
                        nc.tensor.matmul(
                            ps[:],
                            OT[:, pt, qt * P:(qt + 1) * P],
                            wo_sb[:, pt, nch * 512:(nch + 1) * 512],
                            start=(pt == 0), stop=(pt == NPT - 1),
                        )
                    ot = work.tile([P, 512], f32, tag="outstage")
                    evac(ot[:], ps[:])
                    nc.sync.dma_start(
                        out[qt * P:(qt + 1) * P, nch * 512:(nch + 1) * 512],
                        ot[:])

        # --- emission order = scheduler priority: start the exp stream
        # (the bottleneck engine) as early as possible; later-priority
        # proj/outproj matmuls fill the PE's exp-wait gaps. Outproj for
        # chunk c is emitted after attention chunk c+1 so it never
        # preempts the next chunk's scores (which feed ACT), yet still
        # runs at the boosted mid-kernel clock instead of the cold tail.
        for c in range(NQC):
            proj_fill("k", 0, c, sc_copy, deep=True)
        for c in range(NQC):
            proj_fill("v", 0, c, sc_copy, deep=True)
            v_transposes(0, c)
        proj_fill("q", 0, 0, sc_copy, deep=True)
        attn_chunk(0, 0)
        for c in range(1, NQC):
            proj_fill("q", 0, c, ve_copy)
        attn_chunk(0, QC)
        proj_fill("k", 1, 0, ve_copy)
        proj_fill("k", 1, 1, ve_copy)
        attn_chunk(0, 2 * QC)
        proj_fill("k", 1, 2, ve_copy)
        proj_fill("k", 1, 3, ve_copy)
        proj_fill("v", 1, 0, ve_copy)
        v_transposes(1, 0)
        attn_chunk(0, 3 * QC)
        for c in range(1, NQC):
            proj_fill("v", 1, c, ve_copy)
            v_transposes(1, c)
        for c in range(NQC):
            proj_fill("q", 1, c, ve_copy)
        # pair 1: three 512 chunks, then 384 + 128 so the final
        # norm -> outproj chain after the last exp is as short as possible
        attn_chunk(1, 0)
        attn_chunk(1, QC)
        outproj_chunk(range(0, 4))
        attn_chunk(1, 2 * QC)
        outproj_chunk(range(4, 8))
        attn_chunk(1, 3 * QC, 384)
        outproj_chunk(range(8, 12))
        attn_chunk(1, 3 * QC + 384, 128)
        outproj_chunk(range(12, 15), deep=True)
        outproj_chunk([15], deep=True)


def _in_maps(x, Wq, Wk, Wv, Wo):
    in_np = _np_dt(CFG["proj"])
    wo_np = _np_dt(CFG["wo"])
    x = np.asarray(x, dtype=np.float32)
    Wq = np.asarray(Wq, dtype=np.float32)
    Wk = np.asarray(Wk, dtype=np.float32)
    Wv = np.asarray(Wv, dtype=np.float32)
    Wo = np.asarray(Wo, dtype=np.float32)
    xT = [np.ascontiguousarray(x[b].T).astype(in_np) for b in range(B)]
    maps = []
    for c in range(NCORES):
        b, g = c // GROUPS, c % GROUPS
        rows = slice(g * M, (g + 1) * M)
        maps.append({
            "xT": xT[b],
            "wq": np.ascontiguousarray(Wq[rows, :].T).astype(in_np),
            "wk": np.ascontiguousarray(Wk[rows, :].T).astype(in_np),
            "wv": np.ascontiguousarray(Wv[rows, :].T).astype(in_np),
            "wo": np.ascontiguousarray(Wo[:, rows].T).astype(wo_np),
        })
    return maps


def kernel(x, Wq, Wk, Wv, Wo, _trace=False):
    global _compiled
    if _compiled is None:
        _compiled = _build_module()
    from concourse.bass_utils import run_bass_kernel_spmd

    res = run_bass_kernel_spmd(
        _compiled, _in_maps(x, Wq, Wk, Wv, Wo),
        core_ids=list(range(NCORES)), trace=_trace,
    )
    outs = [r["out"] for r in res.results]
    y = np.empty((B, S, D), np.float32)
    for b in range(B):
        y[b] = outs[4 * b] + outs[4 * b + 1] + outs[4 * b + 2] + outs[4 * b + 3]
    if _trace:
        kernel.last_results = res
    return y
